# revision 5
# baseline (speedup 1.0000x reference)
"""AttnBlock (GroupNorm -> single-head attention over 64x64 tokens -> proj -> residual)
for Trainium2, SPMD over 8 NeuronCores.

Sharding: core = batch(4) x query-half(2).  Each core receives x[b] with its
query half rotated to the front (token order along j is permutation-invariant
for softmax-attention and for GroupNorm stats), computes GroupNorm + k/vT over
all 4096 tokens, q over its 2048 tokens, streaming-softmax attention without
max-subtraction (logits bounded ~7), and the output projection + residual for
its 2048 tokens.

All matmuls run in bf16 (fp32 PSUM accumulation); measured end-to-end L2 rel
err vs the fp32 reference ~3e-4.

Layouts (SBUF, partition dim first):
  h, k : [128, 4cc, 4096]  channel on partitions (4 chunks of 128), tokens free
  q    : [128, 4cc, 2048]
  vT   : [128jc, 32, 512]  token chunk on partitions, channel free
  S^T  : psum [128 j, 512 i] = sum_c k[c,j] q[c,i]  (no transposes anywhere)
  O    : psum [128 c, 512 i] = sum_j vT[j,c] * exp(S^T[j,i]), then / l_i
"""

import math
import numpy as np
import ml_dtypes

import concourse.bass as bass
import concourse.mybir as mybir
import concourse.tile as tile

P = 128
C = 512
NCC = C // P          # 4 channel chunks
HW = 4096             # tokens per batch image
IHALF = 2048          # query tokens per core
NBLK = IHALF // 512   # 4 i-blocks of 512
NJC = HW // P         # 32 j chunks of 128
NJT = HW // 512       # 8 j tiles of 512
GS = 16               # channels per group
EPS = 1e-6
INV_SQRT_C = 1.0 / math.sqrt(C)

F32 = mybir.dt.float32
BF16 = mybir.dt.bfloat16
BF = ml_dtypes.bfloat16


def _split_excess_waits(nc):
    """walrus in this container accepts only ONE sync-wait per instruction;
    move extra waits onto same-engine NOPs placed immediately before."""
    for fn in nc.m.functions:
        for bb in fn.blocks:
            insts = list(bb.instructions)
            out = []
            changed = False
            for inst in insts:
                si = inst.sync_info
                if si is not None and len(si.on_wait) > 1:
                    waits = list(si.on_wait)
                    for k, w in enumerate(waits[:-1]):
                        nop = mybir.InstNoOp(
                            name=f"{inst.name}-ws{k}",
                            sync_info=mybir.SyncInfo(on_wait=[w], on_update=[]),
                            bass_nofuse=True,
                            engine=inst.engine,
                        )
                        out.append(nop)
                    inst.sync_info = mybir.SyncInfo(
                        on_wait=[waits[-1]], on_update=list(si.on_update)
                    )
                    changed = True
                out.append(inst)
            if changed:
                bb.instructions = out


def build_nc(split_waits=True):
    nc = bass.Bass()

    x_d = nc.declare_dram_parameter("x_bc", [C, HW], F32, isOutput=False)
    wqt_d = nc.declare_dram_parameter("wqt", [C, C], BF16, isOutput=False)
    wkt_d = nc.declare_dram_parameter("wkt", [C, C], BF16, isOutput=False)
    wvt_d = nc.declare_dram_parameter("wvt", [C, C], BF16, isOutput=False)
    wpt_d = nc.declare_dram_parameter("wpt", [C, C], BF16, isOutput=False)
    bq_d = nc.declare_dram_parameter("bq_pc", [P, NCC], F32, isOutput=False)
    bk_d = nc.declare_dram_parameter("bk_pc", [P, NCC], F32, isOutput=False)
    bp_d = nc.declare_dram_parameter("bp_pc", [P, NCC], F32, isOutput=False)
    gamma_d = nc.declare_dram_parameter("gamma_pc", [P, NCC], F32, isOutput=False)
    beta_d = nc.declare_dram_parameter("beta_pc", [P, NCC], F32, isOutput=False)
    bv_d = nc.declare_dram_parameter("bv_row", [1, C], F32, isOutput=False)
    ind16_d = nc.declare_dram_parameter("ind16", [P, P // GS], F32, isOutput=False)
    bcast16_d = nc.declare_dram_parameter("bcast16", [P // GS, P], F32, isOutput=False)
    ones_d = nc.declare_dram_parameter("ones_col", [P, 1], BF16, isOutput=False)
    y_d = nc.declare_dram_parameter("yout", [C, IHALF], F32, isOutput=True)

    with tile.TileContext(nc) as tc:
        # ---- persistent pools (live through the whole kernel) ----
        with (
            tc.tile_pool(name="w", bufs=1) as wpool,
            tc.tile_pool(name="const", bufs=1) as cpool,
            tc.tile_pool(name="kbuf", bufs=1) as kpool,
            tc.tile_pool(name="vbuf", bufs=1) as vpool,
            tc.tile_pool(name="qbuf", bufs=1) as qpool,
        ):
            wqt = wpool.tile([P, NCC, C], BF16, tag="wqt")
            wkt = wpool.tile([P, NCC, C], BF16, tag="wkt")
            wvt = wpool.tile([P, NCC, C], BF16, tag="wvt")
            wpt = wpool.tile([P, NCC, C], BF16, tag="wpt")
            for t, d in ((wqt, wqt_d), (wkt, wkt_d), (wvt, wvt_d), (wpt, wpt_d)):
                nc.sync.dma_start(out=t[:], in_=d[:].rearrange("(cc p) o -> p cc o", p=P))

            bq_sb = cpool.tile([P, NCC], F32, tag="bq")
            bk_sb = cpool.tile([P, NCC], F32, tag="bk")
            bp_sb = cpool.tile([P, NCC], F32, tag="bp")
            gamma_sb = cpool.tile([P, NCC], F32, tag="gamma")
            beta_sb = cpool.tile([P, NCC], F32, tag="beta")
            ind16_sb = cpool.tile([P, P // GS], F32, tag="ind16")
            bcast16_sb = cpool.tile([P // GS, P], F32, tag="bcast16")
            ones_sb = cpool.tile([P, 1], BF16, tag="ones")
            bv_sb = cpool.tile([P, C], F32, tag="bvb")
            eps_sb = cpool.tile([P // GS, 1], F32, tag="eps")
            for t, d in (
                (bq_sb, bq_d), (bk_sb, bk_d), (bp_sb, bp_d),
                (gamma_sb, gamma_d), (beta_sb, beta_d),
                (ind16_sb, ind16_d), (bcast16_sb, bcast16_d), (ones_sb, ones_d),
            ):
                nc.sync.dma_start(out=t[:], in_=d[:])
            nc.sync.dma_start(out=bv_sb[:], in_=bv_d[:].to_broadcast((P, C)))
            nc.vector.memset(eps_sb[:], EPS)

            k_sb = kpool.tile([P, NCC, HW], BF16, tag="k")
            vt_sb = vpool.tile([P, NJC, C], BF16, tag="vt")
            q_sb = qpool.tile([P, NCC, IHALF], BF16, tag="q")

            # =========== phase 0: GroupNorm statistics ===========
            with (
                tc.tile_pool(name="xs", bufs=2) as xpool,
                tc.tile_pool(name="gn", bufs=2) as gpool,
                tc.tile_pool(name="hbuf", bufs=1) as hpool,
                tc.tile_pool(name="gnp", bufs=2, space="PSUM") as gpsum_pool,
            ):
                h_sb = hpool.tile([P, NCC, HW], BF16, tag="h")

                gpsum = gpsum_pool.tile([P // GS, 2 * NCC], F32, tag="gstat")
                for ci in range(NCC):
                    x_t = xpool.tile([P, HW], F32, tag="x")
                    nc.sync.dma_start(out=x_t[:], in_=x_d[ci * P:(ci + 1) * P, :])
                    stats = gpool.tile([P, HW // 512, 6], F32, tag="stats")
                    for sg in range(HW // 512):
                        nc.vector.bn_stats(
                            out=stats[:, sg, :], in_=x_t[:, sg * 512:(sg + 1) * 512]
                        )
                    mv = gpool.tile([P, 2], F32, tag="mv")
                    nc.vector.bn_aggr(out=mv[:], in_=stats[:])
                    # t = [mean, E[x^2]] per channel
                    t2 = gpool.tile([P, 2], F32, tag="t2")
                    nc.vector.tensor_copy(out=t2[:, 0:1], in_=mv[:, 0:1])
                    nc.vector.tensor_tensor(
                        t2[:, 1:2], mv[:, 0:1], mv[:, 0:1], mybir.AluOpType.mult
                    )
                    nc.vector.tensor_add(t2[:, 1:2], t2[:, 1:2], mv[:, 1:2])
                    # aggregate over the 16-channel groups living in this chunk
                    nc.tensor.matmul(
                        gpsum[:, ci * 2:(ci + 1) * 2], lhsT=ind16_sb[:], rhs=t2[:],
                        start=True, stop=True,
                    )

                # gstats[g, ci, {mu, ex2}] -> per-group mean/rstd
                gstats = gpool.tile([P // GS, NCC, 2], F32, tag="gstats")
                nc.vector.tensor_copy(out=gstats[:], in_=gpsum[:])
                gmr = gpool.tile([P // GS, NCC, 2], F32, tag="gmr")
                for ci in range(NCC):
                    mu = gstats[:, ci, 0:1]
                    ex2 = gstats[:, ci, 1:2]
                    nc.vector.tensor_copy(out=gmr[:, ci, 0:1], in_=mu)
                    var = gmr[:, ci, 1:2]
                    nc.vector.tensor_tensor(var, mu, mu, mybir.AluOpType.mult)
                    nc.vector.tensor_tensor(var, ex2, var, mybir.AluOpType.subtract)
                    nc.scalar.activation(
                        out=var, in_=var, func=mybir.ActivationFunctionType.Sqrt,
                        bias=eps_sb[:], scale=1.0,
                    )
                    nc.vector.reciprocal(out=var, in_=var)

                # broadcast group stats back to channels, fold gamma/beta
                scale_sb = gpool.tile([P, NCC], F32, tag="scale")
                shift_sb = gpool.tile([P, NCC], F32, tag="shift")
                for ci in range(NCC):
                    bpsum = gpsum_pool.tile([P, 2], F32, tag="bc")
                    nc.tensor.matmul(
                        bpsum[:], lhsT=bcast16_sb[:], rhs=gmr[:, ci, :],
                        start=True, stop=True,
                    )
                    sc = scale_sb[:, ci:ci + 1]
                    sh = shift_sb[:, ci:ci + 1]
                    nc.vector.tensor_tensor(
                        sc, bpsum[:, 1:2], gamma_sb[:, ci:ci + 1], mybir.AluOpType.mult
                    )
                    nc.vector.tensor_tensor(sh, bpsum[:, 0:1], sc, mybir.AluOpType.mult)
                    nc.vector.tensor_tensor(
                        sh, beta_sb[:, ci:ci + 1], sh, mybir.AluOpType.subtract
                    )

                # =========== phase 1: h = GN(x); k, vT, q projections ===========
                for ci in range(NCC):
                    x_t = xpool.tile([P, HW], F32, tag="x")
                    nc.sync.dma_start(out=x_t[:], in_=x_d[ci * P:(ci + 1) * P, :])
                    nc.vector.tensor_scalar(
                        out=h_sb[:, ci, :], in0=x_t[:],
                        scalar1=scale_sb[:, ci:ci + 1], scalar2=shift_sb[:, ci:ci + 1],
                        op0=mybir.AluOpType.mult, op1=mybir.AluOpType.add,
                    )

                with tc.tile_pool(name="mmp", bufs=4, space="PSUM") as mmpool:
                    # k[o, j] (all tokens)
                    for oc in range(NCC):
                        for jt in range(NJT):
                            ps = mmpool.tile([P, 512], F32, tag="mm")
                            for cc in range(NCC):
                                nc.tensor.matmul(
                                    ps[:],
                                    lhsT=wkt[:, cc, oc * P:(oc + 1) * P],
                                    rhs=h_sb[:, cc, jt * 512:(jt + 1) * 512],
                                    start=(cc == 0), stop=(cc == NCC - 1),
                                )
                            nc.scalar.activation(
                                out=k_sb[:, oc, jt * 512:(jt + 1) * 512], in_=ps[:],
                                func=mybir.ActivationFunctionType.Identity,
                                bias=bk_sb[:, oc:oc + 1], scale=1.0,
                            )
                    # vT[j, c] (all tokens)
                    for jc in range(NJC):
                        ps = mmpool.tile([P, 512], F32, tag="mm")
                        for cc in range(NCC):
                            nc.tensor.matmul(
                                ps[:],
                                lhsT=h_sb[:, cc, jc * P:(jc + 1) * P],
                                rhs=wvt[:, cc, :],
                                start=(cc == 0), stop=(cc == NCC - 1),
                            )
                        nc.vector.tensor_add(vt_sb[:, jc, :], ps[:], bv_sb[:])
                    # q[o, i] (this core's half)
                    for oc in range(NCC):
                        for it in range(IHALF // 512):
                            ps = mmpool.tile([P, 512], F32, tag="mm")
                            for cc in range(NCC):
                                nc.tensor.matmul(
                                    ps[:],
                                    lhsT=wqt[:, cc, oc * P:(oc + 1) * P],
                                    rhs=h_sb[:, cc, it * 512:(it + 1) * 512],
                                    start=(cc == 0), stop=(cc == NCC - 1),
                                )
                            nc.scalar.activation(
                                out=q_sb[:, oc, it * 512:(it + 1) * 512], in_=ps[:],
                                func=mybir.ActivationFunctionType.Identity,
                                bias=bq_sb[:, oc:oc + 1], scale=1.0,
                            )

            # =========== phase 2: attention + projection per 512-token block ===========
            with (
                tc.tile_pool(name="et", bufs=3) as etpool,
                tc.tile_pool(name="ob", bufs=2) as obpool,
                tc.tile_pool(name="xr", bufs=4) as xrpool,
                tc.tile_pool(name="os", bufs=4) as ospool,
                tc.tile_pool(name="lb", bufs=2) as lbpool,
                tc.tile_pool(name="ld", bufs=2, space="DRAM") as ldpool,
                tc.tile_pool(name="stp", bufs=2, space="PSUM") as stpool,
                tc.tile_pool(name="oap", bufs=1, space="PSUM") as oapool,
                tc.tile_pool(name="lp", bufs=1, space="PSUM") as lpool,
            ):
                for ib in range(NBLK):
                    isl = slice(ib * 512, (ib + 1) * 512)
                    opsum = [
                        oapool.tile([P, 512], F32, tag=f"o{cc}", name=f"opsum{cc}")
                        for cc in range(NCC)
                    ]
                    lpsum = lpool.tile([1, 512], F32, tag="l")
                    ets = [None] * NJC

                    def emit_st(jc):
                        ps = stpool.tile([P, 512], F32, tag="st")
                        for cc in range(NCC):
                            nc.tensor.matmul(
                                ps[:],
                                lhsT=k_sb[:, cc, jc * P:(jc + 1) * P],
                                rhs=q_sb[:, cc, isl],
                                start=(cc == 0), stop=(cc == NCC - 1),
                            )
                        et = etpool.tile([P, 512], BF16, tag="et")
                        nc.scalar.activation(
                            out=et[:], in_=ps[:],
                            func=mybir.ActivationFunctionType.Exp, scale=INV_SQRT_C,
                        )
                        ets[jc] = et

                    def emit_av(jc):
                        et = ets[jc]
                        for cc in range(NCC):
                            nc.tensor.matmul(
                                opsum[cc][:],
                                lhsT=vt_sb[:, jc, cc * P:(cc + 1) * P],
                                rhs=et[:],
                                start=(jc == 0), stop=(jc == NJC - 1),
                            )
                        nc.tensor.matmul(
                            lpsum[:], lhsT=ones_sb[:], rhs=et[:],
                            start=(jc == 0), stop=(jc == NJC - 1),
                        )
                        ets[jc] = None

                    emit_st(0)
                    for jc in range(1, NJC):
                        emit_st(jc)
                        emit_av(jc - 1)
                    emit_av(NJC - 1)

                    # softmax denominator -> broadcast across partitions via DRAM bounce
                    l_sb = lbpool.tile([1, 512], F32, tag="lsb")
                    nc.vector.tensor_copy(out=l_sb[:], in_=lpsum[:])
                    nc.vector.reciprocal(out=l_sb[:], in_=l_sb[:])
                    l_dram = ldpool.tile([1, 512], F32, tag="ldram")
                    nc.sync.dma_start(out=l_dram[:], in_=l_sb[:])
                    lrb = lbpool.tile([P, 512], F32, tag="lrb")
                    nc.sync.dma_start(out=lrb[:], in_=l_dram[:].to_broadcast((P, 512)))

                    o_bf = obpool.tile([P, NCC, 512], BF16, tag="obf")
                    for cc in range(NCC):
                        nc.vector.tensor_tensor(
                            o_bf[:, cc, :], opsum[cc][:], lrb[:], mybir.AluOpType.mult
                        )

                    # out = Wp @ O + bp + x  (this block's tokens)
                    for oc in range(NCC):
                        xr = xrpool.tile([P, 512], F32, tag="xr")
                        nc.sync.dma_start(
                            out=xr[:], in_=x_d[oc * P:(oc + 1) * P, isl]
                        )
                        ps = stpool.tile([P, 512], F32, tag="st")
                        for cc in range(NCC):
                            nc.tensor.matmul(
                                ps[:],
                                lhsT=wpt[:, cc, oc * P:(oc + 1) * P],
                                rhs=o_bf[:, cc, :],
                                start=(cc == 0), stop=(cc == NCC - 1),
                            )
                        ost = ospool.tile([P, 512], F32, tag="ost")
                        nc.scalar.activation(
                            out=ost[:], in_=ps[:],
                            func=mybir.ActivationFunctionType.Identity,
                            bias=bp_sb[:, oc:oc + 1], scale=1.0,
                        )
                        nc.vector.tensor_add(ost[:], ost[:], xr[:])
                        nc.sync.dma_start(out=y_d[oc * P:(oc + 1) * P, isl], in_=ost[:])

    if split_waits:
        _split_excess_waits(nc)
    return nc


_NC = None


def _get_nc():
    global _NC
    if _NC is None:
        _NC = build_nc()
    return _NC


def kernel(x, gamma, beta, Wq, bq, Wk, bk, Wv, bv, Wp, bp):
    x = np.asarray(x, dtype=np.float32)
    B, c, H, W = x.shape
    assert (B, c, H, W) == (4, C, 64, 64)
    nc = _get_nc()

    def pc(v):  # [C] -> [P, NCC]
        return np.ascontiguousarray(np.asarray(v, np.float32).reshape(NCC, P).T)

    ind16 = np.zeros((P, P // GS), np.float32)
    ind16[np.arange(P), np.arange(P) // GS] = 1.0 / GS
    bcast16 = np.zeros((P // GS, P), np.float32)
    bcast16[np.arange(P) // GS, np.arange(P)] = 1.0

    shared = {
        "wqt": np.ascontiguousarray(np.asarray(Wq, np.float32).T).astype(BF),
        "wkt": np.ascontiguousarray(np.asarray(Wk, np.float32).T).astype(BF),
        "wvt": np.ascontiguousarray(np.asarray(Wv, np.float32).T).astype(BF),
        "wpt": np.ascontiguousarray(np.asarray(Wp, np.float32).T).astype(BF),
        "bq_pc": pc(bq), "bk_pc": pc(bk), "bp_pc": pc(bp),
        "gamma_pc": pc(gamma), "beta_pc": pc(beta),
        "bv_row": np.ascontiguousarray(np.asarray(bv, np.float32).reshape(1, C)),
        "ind16": ind16, "bcast16": bcast16,
        "ones_col": np.ones((P, 1), BF),
    }

    xf = x.reshape(B, C, HW)
    in_maps = []
    for core in range(8):
        b, half = divmod(core, 2)
        xb = xf[b]
        if half == 0:
            x_bc = xb
        else:
            x_bc = np.concatenate([xb[:, IHALF:], xb[:, :IHALF]], axis=1)
        in_maps.append({"x_bc": np.ascontiguousarray(x_bc), **shared})

    from concourse.bass_utils import run_bass_kernel_spmd

    res = run_bass_kernel_spmd(nc, in_maps, list(range(8)))

    out = np.empty((B, C, HW), np.float32)
    for core in range(8):
        b, half = divmod(core, 2)
        out[b, :, half * IHALF:(half + 1) * IHALF] = res.results[core]["yout"]
    return out.reshape(B, C, H, W)


# revision 6
# speedup vs baseline: 1.1597x; 1.1597x over previous
"""AttnBlock (GroupNorm -> single-head attention over 64x64 tokens -> proj -> residual)
for Trainium2, SPMD over 8 NeuronCores.

Sharding: core = batch(4) x query-half(2).  Each core receives x[b] with its
query half rotated to the front (token order along j is permutation-invariant
for softmax-attention and for GroupNorm stats), computes GroupNorm + k/vT over
all 4096 tokens, q over its 2048 tokens, streaming-softmax attention without
max-subtraction (logits bounded ~7), and the output projection + residual for
its 2048 tokens.

All matmuls run in bf16 (fp32 PSUM accumulation); measured end-to-end L2 rel
err vs the fp32 reference ~3e-4.

Layouts (SBUF, partition dim first):
  h, k : [128, 4cc, 4096]  channel on partitions (4 chunks of 128), tokens free
  q    : [128, 4cc, 2048]
  vT   : [128jc, 32, 512]  token chunk on partitions, channel free
  S^T  : psum [128 j, 512 i] = sum_c k[c,j] q[c,i]  (no transposes anywhere)
  O    : psum [128 c, 512 i] = sum_j vT[j,c] * exp(S^T[j,i]), then / l_i
"""

import math
import numpy as np
import ml_dtypes

import concourse.bass as bass
import concourse.mybir as mybir
import concourse.tile as tile

P = 128
C = 512
NCC = C // P          # 4 channel chunks
HW = 4096             # tokens per batch image
IHALF = 2048          # query tokens per core
NBLK = IHALF // 512   # 4 i-blocks of 512
NJC = HW // P         # 32 j chunks of 128
NJT = HW // 512       # 8 j tiles of 512
GS = 16               # channels per group
EPS = 1e-6
INV_SQRT_C = 1.0 / math.sqrt(C)

F32 = mybir.dt.float32
BF16 = mybir.dt.bfloat16
BF = ml_dtypes.bfloat16


def _split_excess_waits(nc):
    """walrus in this container accepts only ONE sync-wait per instruction;
    move extra waits onto same-engine NOPs placed immediately before."""
    for fn in nc.m.functions:
        for bb in fn.blocks:
            insts = list(bb.instructions)
            out = []
            changed = False
            for inst in insts:
                si = inst.sync_info
                if si is not None and len(si.on_wait) > 1:
                    waits = list(si.on_wait)
                    for k, w in enumerate(waits[:-1]):
                        nop = mybir.InstNoOp(
                            name=f"{inst.name}-ws{k}",
                            sync_info=mybir.SyncInfo(on_wait=[w], on_update=[]),
                            bass_nofuse=True,
                            engine=inst.engine,
                        )
                        out.append(nop)
                    inst.sync_info = mybir.SyncInfo(
                        on_wait=[waits[-1]], on_update=list(si.on_update)
                    )
                    changed = True
                out.append(inst)
            if changed:
                bb.instructions = out


def build_nc(split_waits=True):
    nc = bass.Bass()

    x_d = nc.declare_dram_parameter("x_bc", [C, HW], F32, isOutput=False)
    wqt_d = nc.declare_dram_parameter("wqt", [C, C], BF16, isOutput=False)
    wkt_d = nc.declare_dram_parameter("wkt", [C, C], BF16, isOutput=False)
    wvt_d = nc.declare_dram_parameter("wvt", [C, C], BF16, isOutput=False)
    wpt_d = nc.declare_dram_parameter("wpt", [C, C], BF16, isOutput=False)
    bq_d = nc.declare_dram_parameter("bq_pc", [P, NCC], F32, isOutput=False)
    bk_d = nc.declare_dram_parameter("bk_pc", [P, NCC], F32, isOutput=False)
    bp_d = nc.declare_dram_parameter("bp_pc", [P, NCC], F32, isOutput=False)
    gamma_d = nc.declare_dram_parameter("gamma_pc", [P, NCC], F32, isOutput=False)
    beta_d = nc.declare_dram_parameter("beta_pc", [P, NCC], F32, isOutput=False)
    bv_d = nc.declare_dram_parameter("bv_row", [1, C], F32, isOutput=False)
    ind16_d = nc.declare_dram_parameter("ind16", [P, P // GS], F32, isOutput=False)
    bcast16_d = nc.declare_dram_parameter("bcast16", [P // GS, P], F32, isOutput=False)
    ones_d = nc.declare_dram_parameter("ones_col", [P, 1], BF16, isOutput=False)
    y_d = nc.declare_dram_parameter("yout", [C, IHALF], F32, isOutput=True)

    with tile.TileContext(nc) as tc:
        # ---- persistent pools (live through the whole kernel) ----
        with (
            tc.tile_pool(name="w", bufs=1) as wpool,
            tc.tile_pool(name="const", bufs=1) as cpool,
            tc.tile_pool(name="kbuf", bufs=1) as kpool,
            tc.tile_pool(name="vbuf", bufs=1) as vpool,
            tc.tile_pool(name="qbuf", bufs=1) as qpool,
        ):
            wqt = wpool.tile([P, NCC, C], BF16, tag="wqt")
            wkt = wpool.tile([P, NCC, C], BF16, tag="wkt")
            wvt = wpool.tile([P, NCC, C], BF16, tag="wvt")
            wpt = wpool.tile([P, NCC, C], BF16, tag="wpt")
            for t, d in ((wqt, wqt_d), (wkt, wkt_d), (wvt, wvt_d), (wpt, wpt_d)):
                nc.sync.dma_start(out=t[:], in_=d[:].rearrange("(cc p) o -> p cc o", p=P))

            bq_sb = cpool.tile([P, NCC], F32, tag="bq")
            bk_sb = cpool.tile([P, NCC], F32, tag="bk")
            bp_sb = cpool.tile([P, NCC], F32, tag="bp")
            gamma_sb = cpool.tile([P, NCC], F32, tag="gamma")
            beta_sb = cpool.tile([P, NCC], F32, tag="beta")
            ind16_sb = cpool.tile([P, P // GS], F32, tag="ind16")
            bcast16_sb = cpool.tile([P // GS, P], F32, tag="bcast16")
            ones_f = cpool.tile([P, 1], F32, tag="onesf")
            bv_sb = cpool.tile([P, C], F32, tag="bvb")
            eps_sb = cpool.tile([P // GS, 1], F32, tag="eps")
            for t, d in (
                (bq_sb, bq_d), (bk_sb, bk_d), (bp_sb, bp_d),
                (gamma_sb, gamma_d), (beta_sb, beta_d),
                (ind16_sb, ind16_d), (bcast16_sb, bcast16_d),
            ):
                nc.sync.dma_start(out=t[:], in_=d[:])
            nc.sync.dma_start(out=bv_sb[:], in_=bv_d[:].to_broadcast((P, C)))
            nc.vector.memset(eps_sb[:], EPS)
            nc.vector.memset(ones_f[:], 1.0)

            k_sb = kpool.tile([P, NCC, HW], BF16, tag="k")
            vt_sb = vpool.tile([P, NJC, C], BF16, tag="vt")
            q_sb = qpool.tile([P, NCC, IHALF], BF16, tag="q")

            # ====== phase 0: stream x once -> GN stats (fp32) + bf16 copy ======
            with (
                tc.tile_pool(name="hbuf", bufs=1) as hpool,
                tc.tile_pool(name="xs", bufs=2) as xpool,
                tc.tile_pool(name="gn", bufs=2) as gpool,
                tc.tile_pool(name="gnp", bufs=2, space="PSUM") as gpsum_pool,
            ):
                # holds bf16(x), overwritten in place by h = x*scale + shift
                h_sb = hpool.tile([P, NCC, HW], BF16, tag="h")

                gpsum = gpsum_pool.tile([P // GS, 2 * NCC], F32, tag="gstat")
                for ci in range(NCC):
                    x_t = xpool.tile([P, HW], F32, tag="x")
                    half = HW // 2
                    nc.sync.dma_start(out=x_t[:, :half], in_=x_d[ci * P:(ci + 1) * P, :half])
                    nc.sync.dma_start(out=x_t[:, half:], in_=x_d[ci * P:(ci + 1) * P, half:])
                    stats = gpool.tile([P, HW // 512, 6], F32, tag="stats")
                    for sg in range(HW // 512):
                        nc.vector.bn_stats(
                            out=stats[:, sg, :], in_=x_t[:, sg * 512:(sg + 1) * 512]
                        )
                    # bf16 copy on GpSimd (1-input runs at line rate there),
                    # frees DVE for bn_stats
                    nc.gpsimd.tensor_copy(out=h_sb[:, ci, :half], in_=x_t[:, :half])
                    nc.gpsimd.tensor_copy(out=h_sb[:, ci, half:], in_=x_t[:, half:])
                    mv = gpool.tile([P, 2], F32, tag="mv")
                    nc.vector.bn_aggr(out=mv[:], in_=stats[:])
                    # t2 = [mean, E[x^2]] per channel
                    t2 = gpool.tile([P, 2], F32, tag="t2")
                    nc.vector.tensor_copy(out=t2[:, 0:1], in_=mv[:, 0:1])
                    nc.vector.tensor_tensor(
                        t2[:, 1:2], mv[:, 0:1], mv[:, 0:1], mybir.AluOpType.mult
                    )
                    nc.vector.tensor_add(t2[:, 1:2], t2[:, 1:2], mv[:, 1:2])
                    # aggregate over the 16-channel groups living in this chunk
                    nc.tensor.matmul(
                        gpsum[:, ci * 2:(ci + 1) * 2], lhsT=ind16_sb[:], rhs=t2[:],
                        start=True, stop=True,
                    )

                # gstats[g, ci, {mu, ex2}] -> per-group mean/rstd
                gstats = gpool.tile([P // GS, NCC, 2], F32, tag="gstats")
                nc.vector.tensor_copy(out=gstats[:], in_=gpsum[:])
                gmr = gpool.tile([P // GS, NCC, 2], F32, tag="gmr")
                for ci in range(NCC):
                    mu = gstats[:, ci, 0:1]
                    ex2 = gstats[:, ci, 1:2]
                    nc.vector.tensor_copy(out=gmr[:, ci, 0:1], in_=mu)
                    var = gmr[:, ci, 1:2]
                    nc.vector.tensor_tensor(var, mu, mu, mybir.AluOpType.mult)
                    nc.vector.tensor_tensor(var, ex2, var, mybir.AluOpType.subtract)
                    nc.scalar.activation(
                        out=var, in_=var, func=mybir.ActivationFunctionType.Sqrt,
                        bias=eps_sb[:], scale=1.0,
                    )
                    nc.vector.reciprocal(out=var, in_=var)

                # broadcast group stats back to channels, fold gamma/beta
                scale_sb = gpool.tile([P, NCC], F32, tag="scale")
                shift_sb = gpool.tile([P, NCC], F32, tag="shift")
                for ci in range(NCC):
                    bpsum = gpsum_pool.tile([P, 2], F32, tag="bc")
                    nc.tensor.matmul(
                        bpsum[:], lhsT=bcast16_sb[:], rhs=gmr[:, ci, :],
                        start=True, stop=True,
                    )
                    sc = scale_sb[:, ci:ci + 1]
                    sh = shift_sb[:, ci:ci + 1]
                    nc.vector.tensor_tensor(
                        sc, bpsum[:, 1:2], gamma_sb[:, ci:ci + 1], mybir.AluOpType.mult
                    )
                    nc.vector.tensor_tensor(sh, bpsum[:, 0:1], sc, mybir.AluOpType.mult)
                    nc.vector.tensor_tensor(
                        sh, beta_sb[:, ci:ci + 1], sh, mybir.AluOpType.subtract
                    )

                # ====== phase 1: h = x*scale + shift in place; k, vT, q ======
                for ci in range(NCC):
                    # split the affine across DVE and ACT so neither serializes
                    if ci < 2:
                        nc.vector.tensor_scalar(
                            out=h_sb[:, ci, :], in0=h_sb[:, ci, :],
                            scalar1=scale_sb[:, ci:ci + 1], scalar2=shift_sb[:, ci:ci + 1],
                            op0=mybir.AluOpType.mult, op1=mybir.AluOpType.add,
                        )
                    else:
                        nc.scalar.activation(
                            out=h_sb[:, ci, :], in_=h_sb[:, ci, :],
                            func=mybir.ActivationFunctionType.Identity,
                            bias=shift_sb[:, ci:ci + 1], scale=scale_sb[:, ci:ci + 1],
                        )

                with tc.tile_pool(name="mmp", bufs=4, space="PSUM") as mmpool:
                    # k[o, j] (all tokens)
                    for oc in range(NCC):
                        for jt in range(NJT):
                            ps = mmpool.tile([P, 512], F32, tag="mm")
                            for cc in range(NCC):
                                nc.tensor.matmul(
                                    ps[:],
                                    lhsT=wkt[:, cc, oc * P:(oc + 1) * P],
                                    rhs=h_sb[:, cc, jt * 512:(jt + 1) * 512],
                                    start=(cc == 0), stop=(cc == NCC - 1),
                                )
                            nc.scalar.activation(
                                out=k_sb[:, oc, jt * 512:(jt + 1) * 512], in_=ps[:],
                                func=mybir.ActivationFunctionType.Identity,
                                bias=bk_sb[:, oc:oc + 1], scale=1.0,
                            )
                    # vT[j, c] (all tokens)
                    for jc in range(NJC):
                        ps = mmpool.tile([P, 512], F32, tag="mm")
                        for cc in range(NCC):
                            nc.tensor.matmul(
                                ps[:],
                                lhsT=h_sb[:, cc, jc * P:(jc + 1) * P],
                                rhs=wvt[:, cc, :],
                                start=(cc == 0), stop=(cc == NCC - 1),
                            )
                        nc.vector.tensor_add(vt_sb[:, jc, :], ps[:], bv_sb[:])
                    # q[o, i] (this core's half)
                    for oc in range(NCC):
                        for it in range(IHALF // 512):
                            ps = mmpool.tile([P, 512], F32, tag="mm")
                            for cc in range(NCC):
                                nc.tensor.matmul(
                                    ps[:],
                                    lhsT=wqt[:, cc, oc * P:(oc + 1) * P],
                                    rhs=h_sb[:, cc, it * 512:(it + 1) * 512],
                                    start=(cc == 0), stop=(cc == NCC - 1),
                                )
                            nc.scalar.activation(
                                out=q_sb[:, oc, it * 512:(it + 1) * 512], in_=ps[:],
                                func=mybir.ActivationFunctionType.Identity,
                                bias=bq_sb[:, oc:oc + 1], scale=1.0,
                            )

            # ====== phase 2: attention per 512-token block (proj deferred) ======
            with (
                tc.tile_pool(name="et", bufs=4) as etpool,
                tc.tile_pool(name="ob", bufs=NBLK) as obpool,
                tc.tile_pool(name="la", bufs=2) as lapool,
                tc.tile_pool(name="lb", bufs=2) as lbpool,
                tc.tile_pool(name="ld", bufs=2, space="DRAM") as ldpool,
                tc.tile_pool(name="stp", bufs=3, space="PSUM") as stpool,
                tc.tile_pool(name="oap", bufs=1, space="PSUM") as oapool,
                tc.tile_pool(name="lp", bufs=1, space="PSUM") as lpool,
            ):
                o_bfs = []
                for ib in range(NBLK):
                    isl = slice(ib * 512, (ib + 1) * 512)
                    opsum = [
                        oapool.tile([P, 512], F32, tag=f"o{cc}", name=f"opsum{cc}")
                        for cc in range(NCC)
                    ]
                    lacc = lapool.tile([P, 512], F32, tag="lacc")
                    ets = [None] * NJC

                    def emit_st(jc):
                        ps = stpool.tile([P, 512], F32, tag="st")
                        for cc in range(NCC):
                            nc.tensor.matmul(
                                ps[:],
                                lhsT=k_sb[:, cc, jc * P:(jc + 1) * P],
                                rhs=q_sb[:, cc, isl],
                                start=(cc == 0), stop=(cc == NCC - 1),
                            )
                        et = etpool.tile([P, 512], BF16, tag="et")
                        nc.scalar.activation(
                            out=et[:], in_=ps[:],
                            func=mybir.ActivationFunctionType.Exp, scale=INV_SQRT_C,
                        )
                        ets[jc] = et

                    def emit_av(jc):
                        et = ets[jc]
                        for cc in range(NCC):
                            nc.tensor.matmul(
                                opsum[cc][:],
                                lhsT=vt_sb[:, jc, cc * P:(cc + 1) * P],
                                rhs=et[:],
                                start=(jc == 0), stop=(jc == NJC - 1),
                            )
                        # softmax denominator: accumulate exp sums on DVE
                        if jc == 0:
                            nc.vector.tensor_copy(out=lacc[:], in_=et[:])
                        else:
                            nc.vector.tensor_add(lacc[:], lacc[:], et[:])
                        ets[jc] = None

                    DEPTH = 3
                    for jc in range(DEPTH):
                        emit_st(jc)
                    for jc in range(DEPTH, NJC):
                        emit_st(jc)
                        emit_av(jc - DEPTH)
                    for jc in range(NJC - DEPTH, NJC):
                        emit_av(jc)

                    # l = column sums of lacc via a single fp32 matmul
                    lpsum = lpool.tile([1, 512], F32, tag="l")
                    nc.tensor.matmul(
                        lpsum[:], lhsT=ones_f[:], rhs=lacc[:], start=True, stop=True
                    )
                    l_sb = lbpool.tile([1, 512], F32, tag="lsb")
                    nc.vector.tensor_copy(out=l_sb[:], in_=lpsum[:])
                    nc.vector.reciprocal(out=l_sb[:], in_=l_sb[:])
                    l_dram = ldpool.tile([1, 512], F32, tag="ldram")
                    nc.sync.dma_start(out=l_dram[:], in_=l_sb[:])
                    lrb = lbpool.tile([P, 512], F32, tag="lrb")
                    nc.sync.dma_start(out=lrb[:], in_=l_dram[:].to_broadcast((P, 512)))

                    o_bf = obpool.tile([P, NCC, 512], BF16, tag="obf", name=f"o_bf{ib}")
                    for cc in range(NCC):
                        nc.vector.tensor_tensor(
                            o_bf[:, cc, :], opsum[cc][:], lrb[:], mybir.AluOpType.mult
                        )
                    o_bfs.append(o_bf)

                # ====== phase 3: out = Wp @ O + bp + x ======
                with (
                    tc.tile_pool(name="xr", bufs=4) as xrpool,
                    tc.tile_pool(name="os", bufs=4) as ospool,
                ):
                    for ib in range(NBLK):
                        isl = slice(ib * 512, (ib + 1) * 512)
                        o_bf = o_bfs[ib]
                        for oc in range(NCC):
                            xr = xrpool.tile([P, 512], F32, tag="xr")
                            nc.sync.dma_start(
                                out=xr[:], in_=x_d[oc * P:(oc + 1) * P, isl]
                            )
                            ps = stpool.tile([P, 512], F32, tag="st")
                            for cc in range(NCC):
                                nc.tensor.matmul(
                                    ps[:],
                                    lhsT=wpt[:, cc, oc * P:(oc + 1) * P],
                                    rhs=o_bf[:, cc, :],
                                    start=(cc == 0), stop=(cc == NCC - 1),
                                )
                            ost = ospool.tile([P, 512], F32, tag="ost")
                            nc.scalar.activation(
                                out=ost[:], in_=ps[:],
                                func=mybir.ActivationFunctionType.Identity,
                                bias=bp_sb[:, oc:oc + 1], scale=1.0,
                            )
                            nc.vector.tensor_add(ost[:], ost[:], xr[:])
                            nc.sync.dma_start(out=y_d[oc * P:(oc + 1) * P, isl], in_=ost[:])

    if split_waits:
        _split_excess_waits(nc)
    return nc


_NC = None


def _get_nc():
    global _NC
    if _NC is None:
        _NC = build_nc()
    return _NC


def kernel(x, gamma, beta, Wq, bq, Wk, bk, Wv, bv, Wp, bp):
    x = np.asarray(x, dtype=np.float32)
    B, c, H, W = x.shape
    assert (B, c, H, W) == (4, C, 64, 64)
    nc = _get_nc()

    def pc(v):  # [C] -> [P, NCC]
        return np.ascontiguousarray(np.asarray(v, np.float32).reshape(NCC, P).T)

    ind16 = np.zeros((P, P // GS), np.float32)
    ind16[np.arange(P), np.arange(P) // GS] = 1.0 / GS
    bcast16 = np.zeros((P // GS, P), np.float32)
    bcast16[np.arange(P) // GS, np.arange(P)] = 1.0

    shared = {
        "wqt": np.ascontiguousarray(np.asarray(Wq, np.float32).T).astype(BF),
        "wkt": np.ascontiguousarray(np.asarray(Wk, np.float32).T).astype(BF),
        "wvt": np.ascontiguousarray(np.asarray(Wv, np.float32).T).astype(BF),
        "wpt": np.ascontiguousarray(np.asarray(Wp, np.float32).T).astype(BF),
        "bq_pc": pc(bq), "bk_pc": pc(bk), "bp_pc": pc(bp),
        "gamma_pc": pc(gamma), "beta_pc": pc(beta),
        "bv_row": np.ascontiguousarray(np.asarray(bv, np.float32).reshape(1, C)),
        "ind16": ind16, "bcast16": bcast16,
        "ones_col": np.ones((P, 1), BF),
    }

    xf = x.reshape(B, C, HW)
    in_maps = []
    for core in range(8):
        b, half = divmod(core, 2)
        xb = xf[b]
        if half == 0:
            x_bc = xb
        else:
            x_bc = np.concatenate([xb[:, IHALF:], xb[:, :IHALF]], axis=1)
        in_maps.append({"x_bc": np.ascontiguousarray(x_bc), **shared})

    from concourse.bass_utils import run_bass_kernel_spmd

    res = run_bass_kernel_spmd(nc, in_maps, list(range(8)))

    out = np.empty((B, C, HW), np.float32)
    for core in range(8):
        b, half = divmod(core, 2)
        out[b, :, half * IHALF:(half + 1) * IHALF] = res.results[core]["yout"]
    return out.reshape(B, C, H, W)
